# revision 23
# baseline (speedup 1.0000x reference)
"""Grouped-Query Attention (S=2048, NQ=32, NKV=8, D=128, HID=4096) on 8 TRN2 NeuronCores.

Sharding: tensor-parallel over heads. Core c owns KV head c and its G=4
query heads (rows c*512..(c+1)*512 of Wq, c*128..(c+1)*128 of Wk/Wv, and
columns c*512..(c+1)*512 of Wo).  Each core computes a partial output
(row-parallel Wo); the host sums the 8 partials.

All matmuls run in bf16 (1 cycle/row on PE) with fp32 PSUM accumulation.
The schedule is built to keep the Tensor engine continuously fed (p-state)
and to keep the DVE light (it was the bottleneck of v1):

  - stage A streams x once and computes kT/vT (all 4 chunks) plus qT for
    chunks 0-2 only; chunk 3's q projection is deferred into stage B(0)
    as PE "filler" work.
  - v[j,d] tiles come from SBUF->SBUF DMA-XBAR transposes (no PE/DVE).
  - stage B computes scores two key-tiles at a time into a 2-bank PSUM
    tile; ONE wide exp per slot halves the Scalar per-op overhead.
  - softmax row-sum accumulation runs in bf16, alternating DVE / GpSimd
    (two partial accumulators, combined by the PE ones-matmul).
  - 1/denominator uses reciprocal_approx_fast (5x faster than
    nc.vector.reciprocal; ~18 good bits).
  - stage C(t-1) output-projection matmuls are interleaved into stage
    B(t)'s slots as filler, so PE never idles while Scalar runs exps;
    their PSUM->SBUF copies run on GpSimd.
  - ctx matmuls are skewed one slot behind their scores so they never
    wait on the Scalar exp latency.
"""

import os
import sys

import numpy as np
import ml_dtypes

for _p in ("/opt/trn_rl_repo", "/root/.axon_site/_ro/trn_rl_repo"):
    if os.path.isdir(_p) and _p not in sys.path:
        sys.path.insert(0, _p)

import concourse.bass as bass
import concourse.bacc as bacc
import concourse.mybir as mybir
import concourse.tile as tile
from concourse.bass_utils import run_bass_kernel_spmd

P = 128          # partitions / head dim / PE tile
S = 2048         # sequence length
HID = 4096       # hidden dim
NCORES = 8
NH = 4           # q heads per core
DQ = NH * P      # per-core q width (512)
SC = 512         # free-dim chunk (PSUM bank = 512 fp32)
NKT = HID // P   # 32 contraction tiles over hidden
NG = 4           # kt groups (8 kt each)
KPG = NKT // NG  # kt per group (8)
NCH = S // SC    # 4 sequence chunks
NJT = S // P     # 16 key tiles
NOC = HID // SC  # 8 out column chunks
SCALE = float(P) ** -0.5
BF = mybir.dt.bfloat16
F32 = mybir.dt.float32
BFNP = np.dtype(ml_dtypes.bfloat16)

_CACHE = {}


def _build():
    # All inputs are pre-rearranged on the host into partition-major
    # layouts so every DMA slice is >=2KB contiguous per partition
    # (256B-descriptor DMAs take ~13us of queue time per MiB).
    nc = bacc.Bacc(None, target_bir_lowering=False)
    xR = nc.declare_dram_parameter("xR", [P, NCH, NKT, SC], BF, isOutput=False)
    WqR = nc.declare_dram_parameter("WqR", [P, NKT, DQ], BF, isOutput=False)
    WkR = nc.declare_dram_parameter("WkR", [P, NKT, P], BF, isOutput=False)
    WvR = nc.declare_dram_parameter("WvR", [P, NKT, P], BF, isOutput=False)
    bvp = nc.declare_dram_parameter("bvp", [P, 1], F32, isOutput=False)
    WoR = nc.declare_dram_parameter("WoR", [P, NH, HID], BF, isOutput=False)
    out = nc.declare_dram_parameter("out", [S, HID], F32, isOutput=True)

    with tile.TileContext(nc) as tc:
        with (
            tc.tile_pool(name="consts", bufs=1) as consts,
            tc.tile_pool(name="acts", bufs=1) as acts,
            tc.tile_pool(name="xin", bufs=3) as xin,
            tc.tile_pool(name="epool", bufs=4) as epool,
            tc.tile_pool(name="rpool", bufs=2) as rpool,
            tc.tile_pool(name="opool", bufs=4) as opool,
        ):
            # ---- constants ----
            ones_bf = consts.tile([P, P], BF)
            nc.vector.memset(ones_bf, 1.0)
            bv_sb = consts.tile([P, 1], F32)
            nc.sync.dma_start(out=bv_sb, in_=bvp[:, :])
            # weight-group DMAs are emitted inside chunk 0's group loop so
            # the sync-queue FIFO pipelines them with compute
            wk = consts.tile([P, NKT, P], BF)
            wv = consts.tile([P, NKT, P], BF)
            wq = consts.tile([P, NKT, DQ], BF)
            wo = consts.tile([P, NH, HID], BF)

            # ---- persistent activations (bf16) ----
            qT = acts.tile([P, NH, S], BF)      # per head: [128 d, 2048 s]
            kT = acts.tile([P, S], BF)          # [128 d, 2048 s]
            vT = acts.tile([P, S], BF)          # [128 d, 2048 s]
            v = acts.tile([P, NJT, P], BF)      # [128 j, jt, 128 d]
            ctxT = acts.tile([P, NH, S], BF)    # per head: [128 d, 2048 i]
            x3 = acts.tile([P, NKT, SC], BF)    # chunk-3 x, kept for q3 filler

            # ---- PE warmup: keep TensorE busy during initial weight DMAs so
            # the p-state ramp completes before real matmuls start ----
            with tc.tile_pool(name="pwarm", bufs=1, space="PSUM") as pwarm:
                wt = pwarm.tile([P, P], F32, name="warm")
                for _ in range(56):
                    nc.tensor.matmul(wt, lhsT=ones_bf, rhs=ones_bf,
                                     start=True, stop=True)

            # ---- stage A, phase 1: k/v projections for ALL chunks.
            # Only ~2 MiB of weights needed up front, so the sync queue can
            # keep pace with compute from the start.  wq/wo/x3... prefetch
            # rides the Activation DGE queue meanwhile. ----
            with tc.tile_pool(name="pacc", bufs=1, space="PSUM") as pacc:
                for c in range(NCH):
                    s0 = c * SC
                    k_ps = pacc.tile([P, SC], F32, tag="pk", bufs=2)
                    v_ps = pacc.tile([P, SC], F32, tag="pv", bufs=2)
                    for g in range(NG):
                        ks = slice(g * KPG, (g + 1) * KPG)
                        if c == NCH - 1:
                            xt = x3[:, ks, :]
                            nc.sync.dma_start(out=x3[:, ks, :],
                                              in_=xR[:, 3, ks, :])
                        else:
                            xt = xin.tile([P, KPG, SC], BF, name="xt")
                            nc.sync.dma_start(out=xt, in_=xR[:, c, ks, :])
                        if c == 0:
                            # k/v weight group g rides right behind its x
                            nc.sync.dma_start(out=wk[:, ks, :],
                                              in_=WkR[:, ks, :])
                            nc.sync.dma_start(out=wv[:, ks, :],
                                              in_=WvR[:, ks, :])
                            # wq on the Activation queue (needed at phase 2)
                            nc.scalar.dma_start(out=wq[:, ks, :],
                                                in_=WqR[:, ks, :])
                        elif c == 1:
                            nc.scalar.dma_start(out=wo[:, g, :],
                                                in_=WoR[:, g, :])
                        for kk in range(KPG):
                            kt = g * KPG + kk
                            st, sp = kt == 0, kt == NKT - 1
                            nc.tensor.matmul(k_ps, lhsT=wk[:, kt, :],
                                             rhs=xt[:, kk, :], start=st, stop=sp)
                        for kk in range(KPG):
                            kt = g * KPG + kk
                            st, sp = kt == 0, kt == NKT - 1
                            nc.tensor.matmul(v_ps, lhsT=wv[:, kt, :],
                                             rhs=xt[:, kk, :], start=st, stop=sp)
                    nc.vector.tensor_copy(out=kT[:, s0:s0 + SC], in_=k_ps)
                    # v = x @ Wv.T + bv  (bias is per-partition in [d, s])
                    nc.scalar.activation(out=vT[:, s0:s0 + SC], in_=v_ps,
                                         func=mybir.ActivationFunctionType.Identity,
                                         bias=bv_sb, scale=1.0)
                    # v[j, d] via DMA-XBAR transpose (no PE/DVE work),
                    # on the Activation DGE queue to keep sync free for x
                    for jj in range(SC // P):
                        jt = c * (SC // P) + jj
                        nc.scalar.dma_start(out=v[:, jt, :],
                                            in_=vT[:, jt * P:(jt + 1) * P],
                                            transpose=True)
                # ---- stage A, phase 2: q projections for chunks 0-2
                # (chunk 3's q is deferred into stage B(0) as filler).
                # x is re-streamed; demand is only ~146 GB/s here. ----
                for c in range(NCH - 1):
                    s0 = c * SC
                    q_ps = [pacc.tile([P, SC], F32, tag="pq%d" % m,
                                      name="q_ps%d" % m)
                            for m in range(NH)]
                    for g in range(NG):
                        ks = slice(g * KPG, (g + 1) * KPG)
                        xt = xin.tile([P, KPG, SC], BF, name="xt")
                        nc.sync.dma_start(out=xt, in_=xR[:, c, ks, :])
                        for m in range(NH):
                            for kk in range(KPG):
                                kt = g * KPG + kk
                                st, sp = kt == 0, kt == NKT - 1
                                nc.tensor.matmul(
                                    q_ps[m],
                                    lhsT=wq[:, kt, m * P:(m + 1) * P],
                                    rhs=xt[:, kk, :], start=st, stop=sp)
                    for m in range(NH):
                        nc.vector.tensor_copy(out=qT[:, m, s0:s0 + SC],
                                              in_=q_ps[m])

            # ---- stages B+C: attention with interleaved filler ----
            # B(t) slots: 4 heads x 8 wide (2-key-tile) slots = 32 slots.
            # Filler per slot: t==0 -> 4 q3-projection matmuls;
            #                  t>=1 -> one C(t-1) group (4 matmuls + copy).
            NSL = NJT // 2  # 8 wide slots per head
            with tc.tile_pool(name="pbc", bufs=1, space="PSUM") as pbc:
                for t in range(NCH):
                    i0 = t * SC
                    for h in range(NH):
                        ctx_ps = pbc.tile([P, SC], F32, tag="pctx", bufs=2)
                        racc_d = rpool.tile([P, SC], BF, tag="racc_d",
                                            name="racc_d")
                        racc_g = rpool.tile([P, SC], BF, tag="racc_g",
                                            name="racc_g")
                        pend_ctx = None  # skewed: ctx for previous slot
                        q3_ps = None
                        if t == 0:
                            q3_ps = pbc.tile([P, SC], F32, tag="pfill",
                                             bufs=2, name="q3_ps")
                        for jp in range(NSL):
                            jt0, jt1 = 2 * jp, 2 * jp + 1
                            s_wide = pbc.tile([P, 2 * SC], F32, tag="psw",
                                              bufs=2, name="s_wide")
                            nc.tensor.matmul(s_wide[:, :SC],
                                             lhsT=kT[:, jt0 * P:(jt0 + 1) * P],
                                             rhs=qT[:, h, i0:i0 + SC],
                                             start=True, stop=True)
                            nc.tensor.matmul(s_wide[:, SC:],
                                             lhsT=kT[:, jt1 * P:(jt1 + 1) * P],
                                             rhs=qT[:, h, i0:i0 + SC],
                                             start=True, stop=True)
                            e_wide = epool.tile([P, 2 * SC], BF, name="e_wide")
                            nc.scalar.activation(
                                out=e_wide, in_=s_wide,
                                func=mybir.ActivationFunctionType.Exp,
                                scale=SCALE)
                            # ---- filler matmuls (keep PE busy while exp runs)
                            if t == 0:
                                for kk in range(4):
                                    kt = jp * 4 + kk
                                    nc.tensor.matmul(
                                        q3_ps,
                                        lhsT=wq[:, kt, h * P:(h + 1) * P],
                                        rhs=x3[:, kt, :],
                                        start=kt == 0, stop=kt == NKT - 1)
                            else:
                                mt = 4 * (t - 1) + h
                                oc = jp
                                m0, o0 = mt * P, oc * SC
                                o_ps = pbc.tile([P, SC], F32, tag="pfill",
                                                bufs=2, name="o_ps")
                                for dt_ in range(NH):
                                    nc.tensor.matmul(
                                        o_ps,
                                        lhsT=ctxT[:, dt_, m0:m0 + P],
                                        rhs=wo[:, dt_, o0:o0 + SC],
                                        start=dt_ == 0, stop=dt_ == NH - 1)
                                ob = opool.tile([P, SC], F32, name="ob")
                                nc.vector.tensor_copy(out=ob, in_=o_ps)
                                nc.sync.dma_start(out=out[m0:m0 + P,
                                                          o0:o0 + SC], in_=ob)
                            # ---- skewed ctx matmuls (previous slot's exp)
                            if pend_ctx is not None:
                                pe, pj0, pj1 = pend_ctx
                                nc.tensor.matmul(ctx_ps, lhsT=v[:, pj0, :],
                                                 rhs=pe[:, :SC],
                                                 start=pj0 == 0, stop=False)
                                nc.tensor.matmul(ctx_ps, lhsT=v[:, pj1, :],
                                                 rhs=pe[:, SC:],
                                                 start=False, stop=False)
                            pend_ctx = (e_wide, jt0, jt1)
                            # ---- row-sum accumulation: alternate DVE/GpSimd.
                            # The LAST slot skips the vector add entirely --
                            # its e-tile feeds the ones-matmul directly, so
                            # the PE never waits on a trailing vector chain.
                            if jp == NSL - 1:
                                e_last = e_wide
                            else:
                                eng = nc.vector if jp % 2 == 0 else nc.gpsimd
                                acc = racc_d if jp % 2 == 0 else racc_g
                                if jp < 2:
                                    eng.tensor_copy(out=acc, in_=e_wide[:, :SC])
                                else:
                                    eng.tensor_add(out=acc, in0=acc,
                                                   in1=e_wide[:, :SC])
                                eng.tensor_add(out=acc, in0=acc,
                                               in1=e_wide[:, SC:])
                        # q3 copy first so the next head's filler unblocks
                        if t == 0:
                            nc.vector.tensor_copy(
                                out=qT[:, h, 3 * SC:4 * SC], in_=q3_ps)
                        # drain the skewed ctx
                        pe, pj0, pj1 = pend_ctx
                        nc.tensor.matmul(ctx_ps, lhsT=v[:, pj0, :],
                                         rhs=pe[:, :SC], start=False, stop=False)
                        nc.tensor.matmul(ctx_ps, lhsT=v[:, pj1, :],
                                         rhs=pe[:, SC:], start=False, stop=True)
                        # denominator: cross-partition sum + broadcast via
                        # ones-matmul over both partial accumulators
                        rb_ps = pbc.tile([P, SC], F32, tag="pfill", bufs=2,
                                         name="rb_ps")
                        nc.tensor.matmul(rb_ps, lhsT=ones_bf, rhs=racc_d,
                                         start=True, stop=False)
                        nc.tensor.matmul(rb_ps, lhsT=ones_bf, rhs=racc_g,
                                         start=False, stop=False)
                        nc.tensor.matmul(rb_ps, lhsT=ones_bf, rhs=e_last[:, :SC],
                                         start=False, stop=False)
                        nc.tensor.matmul(rb_ps, lhsT=ones_bf, rhs=e_last[:, SC:],
                                         start=False, stop=True)
                        rbc = rpool.tile([P, SC], F32, tag="rbc", name="rbc")
                        nc.vector.reciprocal_approx_fast(out=rbc, in_=rb_ps)
                        nc.vector.tensor_mul(out=ctxT[:, h, i0:i0 + SC],
                                             in0=ctx_ps, in1=rbc)
                # ---- C(3) tail: last chunk's output projection.  o_ps
                # alternates between the pfill tag and the (now idle) psw
                # tag for a 4-deep PSUM rotation; copies alternate
                # Scalar / DVE so neither trails the PE stream. ----
                for h in range(NH):
                    for oc in range(NOC):
                        mt = 4 * (NCH - 1) + h
                        m0, o0 = mt * P, oc * SC
                        gi = h * NOC + oc
                        o_ps = pbc.tile([P, SC], F32,
                                        tag="pfill" if gi % 2 == 0 else "psw",
                                        bufs=2, name="o_ps")
                        for dt_ in range(NH):
                            nc.tensor.matmul(o_ps,
                                             lhsT=ctxT[:, dt_, m0:m0 + P],
                                             rhs=wo[:, dt_, o0:o0 + SC],
                                             start=dt_ == 0, stop=dt_ == NH - 1)
                        ob = opool.tile([P, SC], F32, name="ob")
                        if gi % 2 == 0:
                            nc.scalar.activation(
                                out=ob, in_=o_ps,
                                func=mybir.ActivationFunctionType.Identity,
                                scale=1.0)
                        else:
                            nc.vector.tensor_copy(out=ob, in_=o_ps)
                        nc.sync.dma_start(out=out[m0:m0 + P, o0:o0 + SC],
                                          in_=ob)
    nc.finalize()
    return nc


def _get_program():
    if "nc" not in _CACHE:
        _CACHE["nc"] = _build()
    return _CACHE["nc"]


def _prep_inputs(hidden_states, Wq, Wk, Wv, bv, Wo):
    x = np.asarray(hidden_states, np.float32).reshape(S, HID)
    # xR[p, c, kt, sc] = x[c*SC+sc, kt*P+p]
    xR = np.ascontiguousarray(
        x.reshape(NCH, SC, NKT, P).transpose(3, 0, 2, 1)).astype(BFNP)
    Wq = np.asarray(Wq, np.float32)
    Wk = np.asarray(Wk, np.float32)
    Wv = np.asarray(Wv, np.float32)
    bv = np.asarray(bv, np.float32)
    Wo = np.asarray(Wo, np.float32)
    maps = []
    for c in range(NCORES):
        qs = slice(c * DQ, (c + 1) * DQ)
        ks = slice(c * P, (c + 1) * P)
        # W*T [HID, d] -> [p, kt, d] with hid = kt*P + p
        wqT = Wq[qs].T.reshape(NKT, P, DQ).transpose(1, 0, 2)
        wkT = Wk[ks].T.reshape(NKT, P, P).transpose(1, 0, 2)
        wvT = Wv[ks].T.reshape(NKT, P, P).transpose(1, 0, 2)
        # WoT [DQ, HID] -> [p, dt, o] with dq = dt*P + p
        woT = Wo[:, qs].T.reshape(NH, P, HID).transpose(1, 0, 2)
        maps.append({
            "xR": xR,
            "WqR": np.ascontiguousarray(wqT).astype(BFNP),
            "WkR": np.ascontiguousarray(wkT).astype(BFNP),
            "WvR": np.ascontiguousarray(wvT).astype(BFNP),
            "bvp": np.ascontiguousarray(bv[ks]).reshape(P, 1),
            "WoR": np.ascontiguousarray(woT).astype(BFNP),
        })
    return maps


def kernel(hidden_states, Wq, Wk, Wv, bv, Wo, _trace=False, **kw):
    nc = _get_program()
    maps = _prep_inputs(hidden_states, Wq, Wk, Wv, bv, Wo)
    res = run_bass_kernel_spmd(nc, maps, list(range(NCORES)), trace=_trace, **kw)
    out = np.zeros((S, HID), np.float32)
    for c in range(NCORES):
        out += np.asarray(res.results[c]["out"], np.float32)
    if _trace:
        return out.reshape(1, S, HID), res
    return out.reshape(1, S, HID)


# revision 25
# speedup vs baseline: 1.0999x; 1.0999x over previous
"""Grouped-Query Attention (S=2048, NQ=32, NKV=8, D=128, HID=4096) on 8 TRN2 NeuronCores.

Sharding: tensor-parallel over heads. Core c owns KV head c and its G=4
query heads (rows c*512..(c+1)*512 of Wq, c*128..(c+1)*128 of Wk/Wv, and
columns c*512..(c+1)*512 of Wo).  Each core computes a partial output
(row-parallel Wo); the host sums the 8 partials.

All matmuls run in bf16 (1 cycle/row on PE) with fp32 PSUM accumulation.
The schedule is built to keep the Tensor engine continuously fed (p-state)
and to keep the DVE light (it was the bottleneck of v1):

  - stage A streams x once and computes kT/vT (all 4 chunks) plus qT for
    chunks 0-2 only; chunk 3's q projection is deferred into stage B(0)
    as PE "filler" work.
  - v[j,d] tiles come from SBUF->SBUF DMA-XBAR transposes (no PE/DVE).
  - stage B computes scores two key-tiles at a time into a 2-bank PSUM
    tile; ONE wide exp per slot halves the Scalar per-op overhead.
  - softmax row-sum accumulation runs in bf16, alternating DVE / GpSimd
    (two partial accumulators, combined by the PE ones-matmul).
  - 1/denominator uses reciprocal_approx_fast (5x faster than
    nc.vector.reciprocal; ~18 good bits).
  - stage C(t-1) output-projection matmuls are interleaved into stage
    B(t)'s slots as filler, so PE never idles while Scalar runs exps;
    their PSUM->SBUF copies run on GpSimd.
  - ctx matmuls are skewed one slot behind their scores so they never
    wait on the Scalar exp latency.
"""

import os
import sys

import numpy as np
import ml_dtypes

for _p in ("/opt/trn_rl_repo", "/root/.axon_site/_ro/trn_rl_repo"):
    if os.path.isdir(_p) and _p not in sys.path:
        sys.path.insert(0, _p)

import concourse.bass as bass
import concourse.bacc as bacc
import concourse.mybir as mybir
import concourse.tile as tile
from concourse.bass_utils import run_bass_kernel_spmd

P = 128          # partitions / head dim / PE tile
S = 2048         # sequence length
HID = 4096       # hidden dim
NCORES = 8
NH = 4           # q heads per core
DQ = NH * P      # per-core q width (512)
SC = 512         # free-dim chunk (PSUM bank = 512 fp32)
NKT = HID // P   # 32 contraction tiles over hidden
NG = 4           # kt groups (8 kt each)
KPG = NKT // NG  # kt per group (8)
NCH = S // SC    # 4 sequence chunks
NJT = S // P     # 16 key tiles
NOC = HID // SC  # 8 out column chunks
SCALE = float(P) ** -0.5
BF = mybir.dt.bfloat16
F32 = mybir.dt.float32
BFNP = np.dtype(ml_dtypes.bfloat16)

_CACHE = {}


def _build():
    # All inputs are pre-rearranged on the host into partition-major
    # layouts so every DMA slice is >=2KB contiguous per partition
    # (256B-descriptor DMAs take ~13us of queue time per MiB).
    nc = bacc.Bacc(None, target_bir_lowering=False)
    xR = nc.declare_dram_parameter("xR", [P, NCH, NKT, SC], BF, isOutput=False)
    WqR = nc.declare_dram_parameter("WqR", [P, NKT, DQ], BF, isOutput=False)
    WkR = nc.declare_dram_parameter("WkR", [P, NKT, P], BF, isOutput=False)
    WvR = nc.declare_dram_parameter("WvR", [P, NKT, P], BF, isOutput=False)
    bvp = nc.declare_dram_parameter("bvp", [P, 1], F32, isOutput=False)
    WoR = nc.declare_dram_parameter("WoR", [P, NH, HID], BF, isOutput=False)
    out = nc.declare_dram_parameter("out", [S, HID], F32, isOutput=True)

    with tile.TileContext(nc) as tc:
        with (
            tc.tile_pool(name="consts", bufs=1) as consts,
            tc.tile_pool(name="acts", bufs=1) as acts,
            tc.tile_pool(name="xin", bufs=3) as xin,
            tc.tile_pool(name="epool", bufs=4) as epool,
            tc.tile_pool(name="rpool", bufs=2) as rpool,
            tc.tile_pool(name="opool", bufs=4) as opool,
        ):
            # ---- constants ----
            ones_bf = consts.tile([P, P], BF)
            nc.vector.memset(ones_bf, 1.0)
            bv_sb = consts.tile([P, 1], F32)
            nc.sync.dma_start(out=bv_sb, in_=bvp[:, :])
            # weight-group DMAs are emitted inside chunk 0's group loop so
            # the sync-queue FIFO pipelines them with compute
            wk = consts.tile([P, NKT, P], BF)
            wv = consts.tile([P, NKT, P], BF)
            wq = consts.tile([P, NKT, DQ], BF)
            wo = consts.tile([P, NH, HID], BF)

            # ---- persistent activations (bf16) ----
            qT = acts.tile([P, NH, S], BF)      # per head: [128 d, 2048 s]
            kT = acts.tile([P, S], BF)          # [128 d, 2048 s]
            vT = acts.tile([P, S], BF)          # [128 d, 2048 s]
            v = acts.tile([P, NJT, P], BF)      # [128 j, jt, 128 d]
            ctxT = acts.tile([P, NH, S], BF)    # per head: [128 d, 2048 i]
            x3 = acts.tile([P, NKT, SC], BF)    # chunk-3 x, kept for q3 filler

            # ---- PE warmup: keep TensorE busy during initial weight DMAs so
            # the p-state ramp completes before real matmuls start ----
            with tc.tile_pool(name="pwarm", bufs=1, space="PSUM") as pwarm:
                wt = pwarm.tile([P, P], F32, name="warm")
                for _ in range(72):
                    nc.tensor.matmul(wt, lhsT=ones_bf, rhs=ones_bf,
                                     start=True, stop=True)

            # ---- stage A: projections (stream x once; q only for chunks
            # 0-2 -- chunk 3's q is deferred into stage B(0) as filler).
            # x rides the sync queue (~98 GB/s steady); all weights ride
            # the Activation DGE queue (chunk 0 needs 6 MiB there, which
            # lands just-in-time per group at ~190 GB/s per queue). ----
            with tc.tile_pool(name="pacc", bufs=1, space="PSUM") as pacc:
                for c in range(NCH):
                    s0 = c * SC
                    has_q = c < NCH - 1
                    k_ps = pacc.tile([P, SC], F32, tag="pk", bufs=2)
                    v_ps = pacc.tile([P, SC], F32, tag="pv", bufs=2)
                    q_ps = [pacc.tile([P, SC], F32, tag="pq%d" % m,
                                      name="q_ps%d" % m)
                            for m in range(NH)] if has_q else None
                    for g in range(NG):
                        ks = slice(g * KPG, (g + 1) * KPG)
                        if c == NCH - 1:
                            xt = x3[:, ks, :]   # DMA'd during chunk 2
                        else:
                            xt = xin.tile([P, KPG, SC], BF, name="xt")
                            nc.sync.dma_start(out=xt, in_=xR[:, c, ks, :])
                            if c == 0:
                                # weight group g just-in-time on scalar DGE
                                nc.scalar.dma_start(out=wk[:, ks, :],
                                                    in_=WkR[:, ks, :])
                                nc.scalar.dma_start(out=wv[:, ks, :],
                                                    in_=WvR[:, ks, :])
                                nc.scalar.dma_start(out=wq[:, ks, :],
                                                    in_=WqR[:, ks, :])
                            elif c == 1:
                                nc.scalar.dma_start(out=wo[:, g, :],
                                                    in_=WoR[:, g, :])
                            elif c == 2:
                                nc.scalar.dma_start(out=x3[:, ks, :],
                                                    in_=xR[:, 3, ks, :])
                        for kk in range(KPG):
                            kt = g * KPG + kk
                            st, sp = kt == 0, kt == NKT - 1
                            nc.tensor.matmul(k_ps, lhsT=wk[:, kt, :],
                                             rhs=xt[:, kk, :], start=st, stop=sp)
                        for kk in range(KPG):
                            kt = g * KPG + kk
                            st, sp = kt == 0, kt == NKT - 1
                            nc.tensor.matmul(v_ps, lhsT=wv[:, kt, :],
                                             rhs=xt[:, kk, :], start=st, stop=sp)
                        if has_q:
                            for m in range(NH):
                                for kk in range(KPG):
                                    kt = g * KPG + kk
                                    st, sp = kt == 0, kt == NKT - 1
                                    nc.tensor.matmul(
                                        q_ps[m],
                                        lhsT=wq[:, kt, m * P:(m + 1) * P],
                                        rhs=xt[:, kk, :], start=st, stop=sp)
                    nc.vector.tensor_copy(out=kT[:, s0:s0 + SC], in_=k_ps)
                    # v = x @ Wv.T + bv  (bias is per-partition in [d, s])
                    nc.scalar.activation(out=vT[:, s0:s0 + SC], in_=v_ps,
                                         func=mybir.ActivationFunctionType.Identity,
                                         bias=bv_sb, scale=1.0)
                    if has_q:
                        for m in range(NH):
                            nc.vector.tensor_copy(out=qT[:, m, s0:s0 + SC],
                                                  in_=q_ps[m])
                    # v[j, d] via DMA-XBAR transpose (no PE/DVE work)
                    for jj in range(SC // P):
                        jt = c * (SC // P) + jj
                        nc.scalar.dma_start(out=v[:, jt, :],
                                            in_=vT[:, jt * P:(jt + 1) * P],
                                            transpose=True)

            # ---- stages B+C: attention with interleaved filler ----
            # B(t) slots: 4 heads x 8 wide (2-key-tile) slots = 32 slots.
            # Filler per slot: t==0 -> 4 q3-projection matmuls;
            #                  t>=1 -> one C(t-1) group (4 matmuls + copy).
            NSL = NJT // 2  # 8 wide slots per head
            with tc.tile_pool(name="pbc", bufs=1, space="PSUM") as pbc:
                for t in range(NCH):
                    i0 = t * SC
                    for h in range(NH):
                        ctx_ps = pbc.tile([P, SC], F32, tag="pctx", bufs=2)
                        racc_d = rpool.tile([P, SC], BF, tag="racc_d",
                                            name="racc_d")
                        racc_g = rpool.tile([P, SC], BF, tag="racc_g",
                                            name="racc_g")
                        pend_ctx = None  # skewed: ctx for previous slot
                        q3_ps = None
                        if t == 0:
                            q3_ps = pbc.tile([P, SC], F32, tag="pfill",
                                             bufs=2, name="q3_ps")
                        for jp in range(NSL):
                            jt0, jt1 = 2 * jp, 2 * jp + 1
                            s_wide = pbc.tile([P, 2 * SC], F32, tag="psw",
                                              bufs=2, name="s_wide")
                            nc.tensor.matmul(s_wide[:, :SC],
                                             lhsT=kT[:, jt0 * P:(jt0 + 1) * P],
                                             rhs=qT[:, h, i0:i0 + SC],
                                             start=True, stop=True)
                            nc.tensor.matmul(s_wide[:, SC:],
                                             lhsT=kT[:, jt1 * P:(jt1 + 1) * P],
                                             rhs=qT[:, h, i0:i0 + SC],
                                             start=True, stop=True)
                            e_wide = epool.tile([P, 2 * SC], BF, name="e_wide")
                            nc.scalar.activation(
                                out=e_wide, in_=s_wide,
                                func=mybir.ActivationFunctionType.Exp,
                                scale=SCALE)
                            # ---- filler matmuls (keep PE busy while exp runs)
                            if t == 0:
                                for kk in range(4):
                                    kt = jp * 4 + kk
                                    nc.tensor.matmul(
                                        q3_ps,
                                        lhsT=wq[:, kt, h * P:(h + 1) * P],
                                        rhs=x3[:, kt, :],
                                        start=kt == 0, stop=kt == NKT - 1)
                            else:
                                mt = 4 * (t - 1) + h
                                oc = jp
                                m0, o0 = mt * P, oc * SC
                                o_ps = pbc.tile([P, SC], F32, tag="pfill",
                                                bufs=2, name="o_ps")
                                for dt_ in range(NH):
                                    nc.tensor.matmul(
                                        o_ps,
                                        lhsT=ctxT[:, dt_, m0:m0 + P],
                                        rhs=wo[:, dt_, o0:o0 + SC],
                                        start=dt_ == 0, stop=dt_ == NH - 1)
                                ob = opool.tile([P, SC], F32, name="ob")
                                nc.vector.tensor_copy(out=ob, in_=o_ps)
                                nc.sync.dma_start(out=out[m0:m0 + P,
                                                          o0:o0 + SC], in_=ob)
                            # ---- skewed ctx matmuls (previous slot's exp)
                            if pend_ctx is not None:
                                pe, pj0, pj1 = pend_ctx
                                nc.tensor.matmul(ctx_ps, lhsT=v[:, pj0, :],
                                                 rhs=pe[:, :SC],
                                                 start=pj0 == 0, stop=False)
                                nc.tensor.matmul(ctx_ps, lhsT=v[:, pj1, :],
                                                 rhs=pe[:, SC:],
                                                 start=False, stop=False)
                            pend_ctx = (e_wide, jt0, jt1)
                            # ---- row-sum accumulation: alternate DVE/GpSimd.
                            # The LAST slot skips the vector add entirely --
                            # its e-tile feeds the ones-matmul directly, so
                            # the PE never waits on a trailing vector chain.
                            if jp == NSL - 1:
                                e_last = e_wide
                            else:
                                eng = nc.vector if jp % 2 == 0 else nc.gpsimd
                                acc = racc_d if jp % 2 == 0 else racc_g
                                if jp < 2:
                                    eng.tensor_copy(out=acc, in_=e_wide[:, :SC])
                                else:
                                    eng.tensor_add(out=acc, in0=acc,
                                                   in1=e_wide[:, :SC])
                                eng.tensor_add(out=acc, in0=acc,
                                               in1=e_wide[:, SC:])
                        # q3 copy first so the next head's filler unblocks
                        if t == 0:
                            nc.vector.tensor_copy(
                                out=qT[:, h, 3 * SC:4 * SC], in_=q3_ps)
                        # drain the skewed ctx
                        pe, pj0, pj1 = pend_ctx
                        nc.tensor.matmul(ctx_ps, lhsT=v[:, pj0, :],
                                         rhs=pe[:, :SC], start=False, stop=False)
                        nc.tensor.matmul(ctx_ps, lhsT=v[:, pj1, :],
                                         rhs=pe[:, SC:], start=False, stop=True)
                        # denominator: cross-partition sum + broadcast via
                        # ones-matmul over both partial accumulators
                        rb_ps = pbc.tile([P, SC], F32, tag="pfill", bufs=2,
                                         name="rb_ps")
                        nc.tensor.matmul(rb_ps, lhsT=ones_bf, rhs=racc_d,
                                         start=True, stop=False)
                        nc.tensor.matmul(rb_ps, lhsT=ones_bf, rhs=racc_g,
                                         start=False, stop=False)
                        nc.tensor.matmul(rb_ps, lhsT=ones_bf, rhs=e_last[:, :SC],
                                         start=False, stop=False)
                        nc.tensor.matmul(rb_ps, lhsT=ones_bf, rhs=e_last[:, SC:],
                                         start=False, stop=True)
                        rbc = rpool.tile([P, SC], F32, tag="rbc", name="rbc")
                        nc.vector.reciprocal_approx_fast(out=rbc, in_=rb_ps)
                        nc.vector.tensor_mul(out=ctxT[:, h, i0:i0 + SC],
                                             in0=ctx_ps, in1=rbc)
                # ---- C(3) tail: last chunk's output projection.  o_ps
                # alternates between the pfill tag and the (now idle) psw
                # tag for a 4-deep PSUM rotation; copies alternate
                # Scalar / DVE so neither trails the PE stream. ----
                for h in range(NH):
                    for oc in range(NOC):
                        mt = 4 * (NCH - 1) + h
                        m0, o0 = mt * P, oc * SC
                        gi = h * NOC + oc
                        o_ps = pbc.tile([P, SC], F32,
                                        tag="pfill" if gi % 2 == 0 else "psw",
                                        bufs=2, name="o_ps")
                        for dt_ in range(NH):
                            nc.tensor.matmul(o_ps,
                                             lhsT=ctxT[:, dt_, m0:m0 + P],
                                             rhs=wo[:, dt_, o0:o0 + SC],
                                             start=dt_ == 0, stop=dt_ == NH - 1)
                        ob = opool.tile([P, SC], F32, name="ob")
                        if gi % 2 == 0:
                            nc.scalar.activation(
                                out=ob, in_=o_ps,
                                func=mybir.ActivationFunctionType.Identity,
                                scale=1.0)
                        else:
                            nc.vector.tensor_copy(out=ob, in_=o_ps)
                        nc.sync.dma_start(out=out[m0:m0 + P, o0:o0 + SC],
                                          in_=ob)
    nc.finalize()
    return nc


def _get_program():
    if "nc" not in _CACHE:
        _CACHE["nc"] = _build()
    return _CACHE["nc"]


def _prep_inputs(hidden_states, Wq, Wk, Wv, bv, Wo):
    x = np.asarray(hidden_states, np.float32).reshape(S, HID)
    # xR[p, c, kt, sc] = x[c*SC+sc, kt*P+p]
    xR = np.ascontiguousarray(
        x.reshape(NCH, SC, NKT, P).transpose(3, 0, 2, 1)).astype(BFNP)
    Wq = np.asarray(Wq, np.float32)
    Wk = np.asarray(Wk, np.float32)
    Wv = np.asarray(Wv, np.float32)
    bv = np.asarray(bv, np.float32)
    Wo = np.asarray(Wo, np.float32)
    maps = []
    for c in range(NCORES):
        qs = slice(c * DQ, (c + 1) * DQ)
        ks = slice(c * P, (c + 1) * P)
        # W*T [HID, d] -> [p, kt, d] with hid = kt*P + p
        wqT = Wq[qs].T.reshape(NKT, P, DQ).transpose(1, 0, 2)
        wkT = Wk[ks].T.reshape(NKT, P, P).transpose(1, 0, 2)
        wvT = Wv[ks].T.reshape(NKT, P, P).transpose(1, 0, 2)
        # WoT [DQ, HID] -> [p, dt, o] with dq = dt*P + p
        woT = Wo[:, qs].T.reshape(NH, P, HID).transpose(1, 0, 2)
        maps.append({
            "xR": xR,
            "WqR": np.ascontiguousarray(wqT).astype(BFNP),
            "WkR": np.ascontiguousarray(wkT).astype(BFNP),
            "WvR": np.ascontiguousarray(wvT).astype(BFNP),
            "bvp": np.ascontiguousarray(bv[ks]).reshape(P, 1),
            "WoR": np.ascontiguousarray(woT).astype(BFNP),
        })
    return maps


def kernel(hidden_states, Wq, Wk, Wv, bv, Wo, _trace=False, **kw):
    nc = _get_program()
    maps = _prep_inputs(hidden_states, Wq, Wk, Wv, bv, Wo)
    res = run_bass_kernel_spmd(nc, maps, list(range(NCORES)), trace=_trace, **kw)
    out = np.zeros((S, HID), np.float32)
    for c in range(NCORES):
        out += np.asarray(res.results[c]["out"], np.float32)
    if _trace:
        return out.reshape(1, S, HID), res
    return out.reshape(1, S, HID)
